# revision 3
# baseline (speedup 1.0000x reference)
"""Cross-attention kernel for TRN2, batch-parallel over 8 NeuronCores. v2.

Problem shapes (hardcoded): B=8, C1=C2=256, H=W=32 (S=1024), NH=8, KD=VD=64.

Per-core program (core b computes batch element b, no collectives):
  K1T = (4*Wk1) @ X1 [512, S1] (psum) -> cast fp8e4m3 -> DRAM-bounce remap to
  DoubleRow layout [64, 2, S1] (head a on partitions 0:32, head b on 32:64;
  slot i holds d = 32*i + p).  QK matmuls run fp8 DoubleRow (0.5 cyc/row),
  2x faster than bf16.  Scores psum = 16*s; exp(s/8) = exp(psum/128) split
  between ACT (LUT exp, scale=1/128) and DVE (custom cubic, consts /16^k).
  AV stays bf16 [65, S1] with the ones-column denominator row.  Normalize:
  avs copy (ACT, bf16), reciprocal_approx_fast from av psum (DVE, fp32),
  denominator row broadcast via DRAM bounce DMA, multiply on GPSIMD/DVE.
  Final projection in bf16; y shipped via SBUF copies + DMA.
"""

import sys

for _p in ("/opt/trn_rl_repo", "/root/.axon_site/_ro/trn_rl_repo"):
    if _p not in sys.path:
        sys.path.append(_p)

import numpy as np

import concourse.bass as bass
import concourse.mybir as mybir
import concourse.tile as tile
from concourse import bacc, bass_utils
from concourse import dve_ops
from concourse.dve_spec import AluOp, Bin, One, Spec, Src0
from concourse.dve_spec import C0 as SC0, C1 as SC1, C2 as SC2

F32 = mybir.dt.float32
F32R = mybir.dt.float32r
BF16 = mybir.dt.bfloat16
FP8 = mybir.dt.float8e4

B = 8
C1 = 256
S1 = 1024
C2V = 256
S2 = 1024
NH = 8
KD = 64
VD = 64
P = 128

WS = 4.0  # host-side Wk1/Wk2 scale; scores psum = WS^2 * s

# ---- custom DVE op: one-pass cubic exp(s/8) ---------------------------------
# p(s) = ((s*C2 + C1)*s + C0)*s + 1, consts pre-scaled for raw scores s.
# Fit of exp(x) with intercept 1 on x in [-0.82, 0.82]; max rel err 5e-3.
_EXP_BODY = Bin(
    AluOp.ADD,
    Bin(
        AluOp.MULTIPLY,
        Bin(
            AluOp.ADD,
            Bin(
                AluOp.MULTIPLY,
                Bin(AluOp.ADD, Bin(AluOp.MULTIPLY, Src0, SC2), SC1),
                Src0,
            ),
            SC0,
        ),
        Src0,
    ),
    One,
)
# consts for psum = 16*s (WS^2): divide the raw-score consts by 16, 256, 4096
PC1 = 1.00620561 / 8.0 / 16.0
PC2 = 0.51886828 / 64.0 / 256.0
PC3 = 0.15441254 / 512.0 / 4096.0
ACT_SCALE = 0.125 / 16.0  # exp(psum * 1/128)


def _exp_ref(in0, in1, s0, s1, imm2):
    return ((in0 * imm2 + s1) * in0 + s0) * in0 + 1.0


EXP_OP = dve_ops.DveOp(
    "EXP_CUBIC_ANT",
    Spec(body=_EXP_BODY, reference=_exp_ref),
    subdim=False,
    uops_sha={"v3": "2b376e79438849ef", "v4": "452976e68db449e6"},
)

if EXP_OP.name not in dve_ops._SUB_OPCODE_FOR_NAME:
    dve_ops.OPS.append(EXP_OP)
    dve_ops.CUSTOM_DVE_SPECS[EXP_OP.name] = EXP_OP.spec
    dve_ops._SUB_OPCODE_FOR_NAME[EXP_OP.name] = (
        dve_ops._CUSTOM_DVE_ROW_BASE + len(dve_ops.OPS) - 1
    )

N_WARM_MM = 8


def build_nc(dump=False):
    nc = bacc.Bacc(
        "TRN2",
        target_bir_lowering=False,
        debug=False,
        enable_asserts=False,
        num_devices=B,
    )

    x1 = nc.dram_tensor("x1", [C1, S1], BF16, kind="ExternalInput").ap()
    x2 = nc.dram_tensor("x2", [C2V, S2], BF16, kind="ExternalInput").ap()
    wkv = nc.dram_tensor("wkv", [3, C1, NH * KD], BF16, kind="ExternalInput").ap()
    wot = nc.dram_tensor("wot", [NH * VD, C1], BF16, kind="ExternalInput").ap()
    y = nc.dram_tensor("y", [C1, S1], F32, kind="ExternalOutput").ap()

    with tile.TileContext(nc) as tc:
        with (
            tc.tile_pool(name="const", bufs=1) as cpool,
            tc.tile_pool(name="k8", bufs=2) as k8pool,
            tc.tile_pool(name="expt", bufs=8) as epool,
            tc.tile_pool(name="norm", bufs=4) as npool,
            tc.tile_pool(name="rcpp", bufs=4) as rpool,
            tc.tile_pool(name="yout", bufs=2) as ypool,
            tc.tile_pool(name="pmm", bufs=2, space="PSUM") as pmm,
            tc.tile_pool(name="pav", bufs=2, space="PSUM") as pav,
            tc.tile_pool(name="dscr", bufs=8, space="DRAM") as dpool,
        ):
            # ---- load inputs (parallel queues; wk1 lands first) ----
            x1_big = cpool.tile([P, 2, S1], BF16, name="x1_big")
            x2_big = cpool.tile([P, 2, S2], BF16, name="x2_big")
            wkv_sb = cpool.tile([P, 3, 2, 512], BF16, name="wkv_sb")
            wot_big = cpool.tile([64, NH, C1], BF16, name="wot_big")
            wkv_r = wkv.rearrange("t (c p) f -> p t c f", p=P)
            nc.sync.dma_start(wkv_sb[:, 0], wkv_r[:, 0])
            nc.sync.dma_start(x1_big[:], x1.rearrange("(c p) s -> p c s", p=P))
            nc.gpsimd.dma_start(x2_big[:], x2.rearrange("(c p) s -> p c s", p=P))
            nc.scalar.dma_start(wkv_sb[:, 1], wkv_r[:, 1])
            nc.gpsimd.dma_start(wkv_sb[:, 2], wkv_r[:, 2])
            nc.sync.dma_start(wot_big[:], wot.rearrange("(h r) c -> r h c", r=64))

            x1_sb = [x1_big[:, c, :] for c in range(2)]
            x2_sb = [x2_big[:, c, :] for c in range(2)]
            wk1t_sb = [wkv_sb[:, 0, c, :] for c in range(2)]
            wk2t_sb = [wkv_sb[:, 1, c, :] for c in range(2)]
            wv2t_sb = [wkv_sb[:, 2, c, :] for c in range(2)]
            wot_sb = [wot_big[:, h, :] for h in range(NH)]

            # DoubleRow-layout fp8 K tiles: [64, 2, S] per chunk
            k1tf = [cpool.tile([64, 2, S1], FP8, name=f"k1tf_{m}") for m in range(4)]
            k2tf = [cpool.tile([64, 2, S2], FP8, name=f"k2tf_{m}") for m in range(4)]
            v2a_sb = [
                cpool.tile([P, NH, VD + 1], BF16, name=f"v2a_{s}") for s in range(8)
            ]
            oall_sb = [cpool.tile([64, S1], BF16, name=f"oall_{h}") for h in range(NH)]

            # ---- warmups: ACT exp table + PE HAM release ----
            warm_sb = cpool.tile([P, 512], BF16, name="warm_sb")
            warm_out = cpool.tile([1, 64], BF16, name="warm_out")
            nc.gpsimd.memset(warm_sb[:], 0.0)
            nc.scalar.activation(
                warm_out[:], warm_sb[0:1, 0:64], mybir.ActivationFunctionType.Exp
            )
            if N_WARM_MM:
                wps = pmm.tile([P, S1], F32, tag="qk", name="warm_ps")
                for i in range(N_WARM_MM):
                    nc.tensor.matmul(
                        wps[:, (i % 2) * 512 : (i % 2) * 512 + 512],
                        warm_sb[:, 0:128],
                        warm_sb[:],
                        start=True,
                        stop=True,
                        skip_group_check=True,
                    )

            def emit_proj_chunk(wt_sb, xs_sb, ktf, m, cast_eng, q):
                """ktf[m] ([64, 2, S] fp8 DoubleRow layout) = (wt chunk).T @ xs."""
                ps = pav.tile([P, S1], F32, tag="pav", name=f"pj_{ktf[m].name}")
                for nh_ in range(2):
                    for k in range(2):
                        nc.tensor.matmul(
                            ps[:, nh_ * 512 : (nh_ + 1) * 512],
                            wt_sb[k][:, m * P : (m + 1) * P],
                            xs_sb[k][:, nh_ * 512 : (nh_ + 1) * 512],
                            start=(k == 0),
                            stop=(k == 1),
                        )
                k8 = k8pool.tile([P, S1], FP8, tag="k8", name=f"k8_{ktf[m].name}")
                if cast_eng == "act":
                    nc.scalar.copy(out=k8[:], in_=ps[:])
                else:
                    nc.vector.tensor_copy(out=k8[:], in_=ps[:])
                dsc = dpool.tile([2, 2, 32, S1], FP8, tag="kb", name=f"kb_{ktf[m].name}")
                q.dma_start(dsc[:].rearrange("h i p s -> (h i p) s"), k8[:])
                for hh in range(2):
                    q.dma_start(
                        ktf[m][hh * 32 : (hh + 1) * 32],
                        dsc[hh].rearrange("i p s -> p i s"),
                    )

            def emit_v2_pair(sp):
                ps = pav.tile([P, S1], F32, tag="pav", name=f"pv2_{sp}")
                for half in range(2):
                    s = 2 * sp + half
                    for k in range(2):
                        nc.tensor.matmul(
                            ps[:, half * 512 : (half + 1) * 512],
                            x2_sb[k][:, s * P : (s + 1) * P],
                            wv2t_sb[k][:],
                            start=(k == 0),
                            stop=(k == 1),
                        )
                for half in range(2):
                    s = 2 * sp + half
                    nc.gpsimd.memset(v2a_sb[s][:, :, VD : VD + 1], 1.0)
                    dst = v2a_sb[s][:, :, 0:VD]
                    src = ps[:, half * 512 : (half + 1) * 512].rearrange(
                        "p (h c) -> p h c", c=VD
                    )
                    if half == 0:
                        nc.scalar.copy(out=dst, in_=src)
                    else:
                        nc.vector.tensor_copy(out=dst, in_=src)

            # ---- prologue: K-chunk 0 projections ----
            emit_proj_chunk(wk1t_sb, x1_sb, k1tf, 0, "vec", nc.sync)
            emit_proj_chunk(wk2t_sb, x2_sb, k2tf, 0, "act", nc.gpsimd)

            av_tiles = {}
            et_tiles = {}
            pending = []

            def emit_av(c, s2):
                a, b = 2 * c, 2 * c + 1
                if s2 == 0:
                    for h in (a, b):
                        av_tiles[h] = pav.tile(
                            [VD + 1, S1], F32, tag="pav", name=f"av_{h}"
                        )
                for idx, h in enumerate((a, b)):
                    et = et_tiles.pop((c, s2, idx))
                    for nh_ in range(2):
                        nc.tensor.matmul(
                            av_tiles[h][:, nh_ * 512 : (nh_ + 1) * 512],
                            v2a_sb[s2][:, h, :],
                            et[:, nh_ * 512 : (nh_ + 1) * 512],
                            start=(s2 == 0),
                            stop=(s2 == 7),
                            skip_group_check=True,
                        )

            def emit_normalize(cpair):
                a, b = 2 * cpair, 2 * cpair + 1
                avs = {}
                rcps = {}
                for i, h in enumerate((a, b)):
                    # bf16 copy of AV out of PSUM (for the normalize multiply)
                    avs[h] = npool.tile([VD + 1, S1], BF16, tag="avs", name=f"avs_{h}")
                    nc.scalar.copy(out=avs[h][:], in_=av_tiles[h][:])
                    # fp32 reciprocal straight from PSUM (custom DVE needs
                    # partition base 0; av tiles start at 0)
                    rcps[h] = rpool.tile([VD + 1, S1], F32, tag="rcp", name=f"rcp_{h}")
                    nc.vector.reciprocal_approx_fast(rcps[h][:], av_tiles[h][:])
                # partition-broadcast the reciprocal rows via a DRAM bounce
                reps = {}
                for i, h in enumerate((a, b)):
                    reps[h] = npool.tile([64, S1], F32, tag="rep", name=f"rep_{h}")
                    rdram = dpool.tile([S1], F32, tag="rd", name=f"rd_{h}")
                    q = nc.sync if i == 0 else nc.gpsimd
                    q.dma_start(rdram[:], rcps[h][VD : VD + 1, :])
                    q.dma_start(reps[h][:], rdram[None, :].to_broadcast((64, S1)))
                # oall_h = avs_h[0:64] * rep_h
                for i, h in enumerate((a, b)):
                    if h < 6:
                        nc.gpsimd.tensor_mul(
                            out=oall_sb[h][:], in0=avs[h][0:VD, :], in1=reps[h][:]
                        )
                    else:
                        nc.vector.tensor_mul(
                            out=oall_sb[h][:], in0=avs[h][0:VD, :], in1=reps[h][:]
                        )

            def flush_av(upto):
                while len(pending) > upto:
                    cc, ss = pending.pop(0)
                    emit_av(cc, ss)
                    if ss == 7:
                        emit_normalize(cc)
                        if cc + 2 <= 3:
                            emit_proj_chunk(
                                wk1t_sb, x1_sb, k1tf, cc + 2,
                                "act" if cc == 0 else "vec", nc.sync,
                            )
                            emit_proj_chunk(
                                wk2t_sb, x2_sb, k2tf, cc + 2,
                                "vec" if cc == 0 else "act", nc.gpsimd,
                            )

            def emit_exp(c, s2, idx, qk, eng):
                et = epool.tile([P, S1], BF16, tag="expt", name=f"et_{c}_{s2}_{idx}")
                if eng == "A":
                    nc.scalar.activation(
                        et[:],
                        qk[:],
                        mybir.ActivationFunctionType.Exp,
                        scale=ACT_SCALE,
                    )
                else:
                    nc.vector._custom_dve(
                        EXP_OP, out=et[:], in0=qk[:], s0=PC1, s1=PC2, imm2=PC3
                    )
                et_tiles[(c, s2, idx)] = et

            for c in range(4):
                for s2 in range(8):
                    qks = {}
                    for idx in range(2):  # head idx within the pair
                        base = 32 * idx
                        qk = pmm.tile(
                            [P, S1], F32, tag="qk", name=f"qk_{c}_{s2}_{idx}"
                        )
                        for nh_ in range(2):
                            nc.tensor.matmul(
                                qk[:, nh_ * 512 : (nh_ + 1) * 512],
                                k2tf[c][base : base + 32, :, s2 * P : (s2 + 1) * P],
                                k1tf[c][base : base + 32, :, nh_ * 512 : (nh_ + 1) * 512],
                                start=True,
                                stop=True,
                                perf_mode=mybir.MatmulPerfMode.DoubleRow,
                            )
                        qks[idx] = qk
                    if c == 0:
                        if s2 == 0:
                            emit_v2_pair(0)
                            emit_v2_pair(1)
                        elif s2 == 1:
                            emit_v2_pair(2)
                            emit_v2_pair(3)
                        elif s2 == 2:
                            emit_proj_chunk(wk1t_sb, x1_sb, k1tf, 1, "vec", nc.sync)
                            emit_proj_chunk(wk2t_sb, x2_sb, k2tf, 1, "act", nc.gpsimd)
                    flush_av(2 if c == 0 else 1)
                    # engine split: alternate per step; a few double-ACT steps
                    # to balance ACT(1038)/DVE(1192) per-tile costs
                    if s2 == 3:
                        engs = ("A", "A")
                    elif (s2 + c) % 2 == 0:
                        engs = ("A", "D")
                    else:
                        engs = ("D", "A")
                    for idx in range(2):
                        emit_exp(c, s2, idx, qks[idx], engs[idx])
                    pending.append((c, s2))
            flush_av(0)

            # ---- final projection: y[mt] = sum_h WoT_h.T @ oall_h ----
            fins = {
                mt: pmm.tile([P, S1], F32, tag="qk", name=f"fin_{mt}")
                for mt in range(2)
            }

            def fin_mms(mt, hs):
                for h in hs:
                    for nh_ in range(2):
                        nc.tensor.matmul(
                            fins[mt][:, nh_ * 512 : (nh_ + 1) * 512],
                            wot_sb[h][:, mt * P : (mt + 1) * P],
                            oall_sb[h][:, nh_ * 512 : (nh_ + 1) * 512],
                            start=(h == 0),
                            stop=(h == NH - 1),
                            skip_group_check=True,
                        )

            def ship_y(mt, eng):
                ysb = ypool.tile([P, S1], F32, tag="y", name=f"y_{mt}")
                if eng == "act":
                    nc.scalar.copy(out=ysb[:], in_=fins[mt][:])
                else:
                    nc.vector.tensor_copy(out=ysb[:], in_=fins[mt][:])
                nc.sync.dma_start(y[mt * P : (mt + 1) * P, :], ysb[:])

            fin_mms(0, range(6))
            fin_mms(1, range(6))
            fin_mms(0, (6, 7))
            ship_y(0, "act")
            fin_mms(1, (6, 7))
            ship_y(1, "vec")

    nc.compile()
    return nc


_nc_cache = None


def _get_nc():
    global _nc_cache
    if _nc_cache is None:
        _nc_cache = build_nc()
    return _nc_cache


def _make_in_maps(input1, input2, Wk1, Wk2, Wv2, Wo):
    import ml_dtypes

    bf16 = ml_dtypes.bfloat16
    input1 = np.asarray(input1, dtype=np.float32).astype(bf16)
    input2 = np.asarray(input2, dtype=np.float32).astype(bf16)
    wkv = np.ascontiguousarray(
        np.stack(
            [
                (np.asarray(Wk1, dtype=np.float32) * WS).T.astype(bf16),
                (np.asarray(Wk2, dtype=np.float32) * WS).T.astype(bf16),
                np.asarray(Wv2, dtype=np.float32).T.astype(bf16),
            ]
        )
    )
    wot = np.ascontiguousarray(np.asarray(Wo, dtype=np.float32).T.astype(bf16))
    return [
        {
            "x1": np.ascontiguousarray(input1[b].reshape(C1, S1)),
            "x2": np.ascontiguousarray(input2[b].reshape(C2V, S2)),
            "wkv": wkv,
            "wot": wot,
        }
        for b in range(B)
    ]


def _assemble(results):
    out = np.stack([results[b]["y"] for b in range(B)], axis=0)
    return np.ascontiguousarray(out.reshape(B, C1, 32, 32).astype(np.float32))


def kernel(input1, input2, Wk1, Wk2, Wv2, Wo):
    nc = _get_nc()
    in_maps = _make_in_maps(input1, input2, Wk1, Wk2, Wv2, Wo)
    res = bass_utils.run_bass_kernel_spmd(nc, in_maps, core_ids=list(range(B)))
    return _assemble(res.results)


def kernel_traced(input1, input2, Wk1, Wk2, Wv2, Wo):
    """Like kernel() but with NTFF profiling; returns (out, BassKernelResults)."""
    nc = _get_nc()
    in_maps = _make_in_maps(input1, input2, Wk1, Wk2, Wv2, Wo)
    res = bass_utils.run_bass_kernel_spmd(
        nc, in_maps, core_ids=list(range(B)), trace=True
    )
    return _assemble(res.results), res


# revision 4
# speedup vs baseline: 1.1687x; 1.1687x over previous
"""Cross-attention kernel for TRN2, batch-parallel over 8 NeuronCores. v3.

Problem shapes (hardcoded): B=8, C1=C2=256, H=W=32 (S=1024), NH=8, KD=VD=64.

Per-core program (core b computes batch element b, no collectives):
  K1T = Wk1 @ X1  [512, S1] bf16, K2T likewise; V2 per-head with ones column.
  Attention pair-packed over head pairs (2c, 2c+1): QK matmuls bf16 with the
  head pair on disjoint 64-partition ranges (enables PE row-tile co-issue),
  scores psum held in [128, 1024] tiles so each exp instruction covers a full
  head step (amortizes ACT/DVE init overhead).  exp split between ACT (LUT)
  and DVE (custom cubic EXP_CUBIC_ANT).  AV bf16 [65, S1] with ones-column
  denominator.  Normalize: bf16 avs copy (ACT) + fp32 reciprocal from PSUM
  (DVE) + DRAM-bounce broadcast DMA + GPSIMD/DVE multiplies -> bf16 oall.
  Final projection bf16; AV for step s-1 is emitted before QK for step s so
  the PE never stalls on the exp pipeline.
"""

import sys

for _p in ("/opt/trn_rl_repo", "/root/.axon_site/_ro/trn_rl_repo"):
    if _p not in sys.path:
        sys.path.append(_p)

import numpy as np

import concourse.bass as bass
import concourse.mybir as mybir
import concourse.tile as tile
from concourse import bacc, bass_utils
from concourse import dve_ops
from concourse.dve_spec import AluOp, Bin, One, Spec, Src0
from concourse.dve_spec import C0 as SC0, C1 as SC1, C2 as SC2

F32 = mybir.dt.float32
BF16 = mybir.dt.bfloat16

B = 8
C1 = 256
S1 = 1024
C2V = 256
S2 = 1024
NH = 8
KD = 64
VD = 64
P = 128

# ---- custom DVE op: one-pass cubic exp(s/8) ---------------------------------
_EXP_BODY = Bin(
    AluOp.ADD,
    Bin(
        AluOp.MULTIPLY,
        Bin(
            AluOp.ADD,
            Bin(
                AluOp.MULTIPLY,
                Bin(AluOp.ADD, Bin(AluOp.MULTIPLY, Src0, SC2), SC1),
                Src0,
            ),
            SC0,
        ),
        Src0,
    ),
    One,
)
PC1 = 1.00620561 / 8.0
PC2 = 0.51886828 / 64.0
PC3 = 0.15441254 / 512.0


def _exp_ref(in0, in1, s0, s1, imm2):
    return ((in0 * imm2 + s1) * in0 + s0) * in0 + 1.0


EXP_OP = dve_ops.DveOp(
    "EXP_CUBIC_ANT",
    Spec(body=_EXP_BODY, reference=_exp_ref),
    subdim=False,
    uops_sha={"v3": "2b376e79438849ef", "v4": "452976e68db449e6"},
)

if EXP_OP.name not in dve_ops._SUB_OPCODE_FOR_NAME:
    dve_ops.OPS.append(EXP_OP)
    dve_ops.CUSTOM_DVE_SPECS[EXP_OP.name] = EXP_OP.spec
    dve_ops._SUB_OPCODE_FOR_NAME[EXP_OP.name] = (
        dve_ops._CUSTOM_DVE_ROW_BASE + len(dve_ops.OPS) - 1
    )

N_WARM_MM = 8


def build_nc(dump=False):
    nc = bacc.Bacc(
        "TRN2",
        target_bir_lowering=False,
        debug=False,
        enable_asserts=False,
        num_devices=B,
    )

    x1 = nc.dram_tensor("x1", [C1, S1], BF16, kind="ExternalInput").ap()
    x2 = nc.dram_tensor("x2", [C2V, S2], BF16, kind="ExternalInput").ap()
    wkv = nc.dram_tensor("wkv", [3, C1, NH * KD], BF16, kind="ExternalInput").ap()
    wot = nc.dram_tensor("wot", [NH * VD, C1], BF16, kind="ExternalInput").ap()
    y = nc.dram_tensor("y", [C1, S1], F32, kind="ExternalOutput").ap()

    with tile.TileContext(nc) as tc:
        with (
            tc.tile_pool(name="const", bufs=1) as cpool,
            tc.tile_pool(name="expt", bufs=8) as epool,
            tc.tile_pool(name="norm", bufs=4) as npool,
            tc.tile_pool(name="rcpp", bufs=4) as rpool,
            tc.tile_pool(name="yout", bufs=2) as ypool,
            tc.tile_pool(name="pmm", bufs=2, space="PSUM") as pmm,
            tc.tile_pool(name="pav", bufs=2, space="PSUM") as pav,
            tc.tile_pool(name="dscr", bufs=4, space="DRAM") as dpool,
        ):
            # ---- load inputs (parallel queues; wk1 lands first) ----
            x1_big = cpool.tile([P, 2, S1], BF16, name="x1_big")
            x2_big = cpool.tile([P, 2, S2], BF16, name="x2_big")
            wkv_sb = cpool.tile([P, 3, 2, 512], BF16, name="wkv_sb")
            wot_big = cpool.tile([64, NH, C1], BF16, name="wot_big")
            wkv_r = wkv.rearrange("t (c p) f -> p t c f", p=P)
            nc.sync.dma_start(wkv_sb[:, 0], wkv_r[:, 0])
            nc.sync.dma_start(x1_big[:], x1.rearrange("(c p) s -> p c s", p=P))
            nc.gpsimd.dma_start(x2_big[:], x2.rearrange("(c p) s -> p c s", p=P))
            nc.scalar.dma_start(wkv_sb[:, 1], wkv_r[:, 1])
            nc.gpsimd.dma_start(wkv_sb[:, 2], wkv_r[:, 2])
            nc.sync.dma_start(wot_big[:], wot.rearrange("(h r) c -> r h c", r=64))

            x1_sb = [x1_big[:, c, :] for c in range(2)]
            x2_sb = [x2_big[:, c, :] for c in range(2)]
            wk1t_sb = [wkv_sb[:, 0, c, :] for c in range(2)]
            wk2t_sb = [wkv_sb[:, 1, c, :] for c in range(2)]
            wv2t_sb = [wkv_sb[:, 2, c, :] for c in range(2)]
            wot_sb = [wot_big[:, h, :] for h in range(NH)]

            k1t_sb = [cpool.tile([P, S1], BF16, name=f"k1t_{m}") for m in range(4)]
            k2t_sb = [cpool.tile([P, S2], BF16, name=f"k2t_{m}") for m in range(4)]
            v2a_sb = [
                cpool.tile([P, NH, VD + 1], BF16, name=f"v2a_{s}") for s in range(8)
            ]
            oall_sb = [cpool.tile([64, S1], BF16, name=f"oall_{h}") for h in range(NH)]

            # ---- warmups: ACT exp table + PE HAM release ----
            warm_sb = cpool.tile([P, 512], BF16, name="warm_sb")
            warm_out = cpool.tile([1, 64], BF16, name="warm_out")
            nc.gpsimd.memset(warm_sb[:], 0.0)
            nc.scalar.activation(
                warm_out[:], warm_sb[0:1, 0:64], mybir.ActivationFunctionType.Exp
            )
            if N_WARM_MM:
                wps = pmm.tile([P, S1], F32, tag="qk", name="warm_ps")
                for i in range(N_WARM_MM):
                    nc.tensor.matmul(
                        wps[:, (i % 2) * 512 : (i % 2) * 512 + 512],
                        warm_sb[:, 0:128],
                        warm_sb[:],
                        start=True,
                        stop=True,
                        skip_group_check=True,
                    )

            def emit_proj_chunk(wt_sb, xs_sb, kt, m, cast_eng):
                """kt[m] (bf16 [128, S]) = (wt chunk).T @ xs.  k-outer order so
                the stationary (wt chunk) is loaded once per k."""
                ps = pav.tile([P, S1], F32, tag="pav", name=f"pj_{kt[m].name}")
                for k in range(2):
                    for nh_ in range(2):
                        nc.tensor.matmul(
                            ps[:, nh_ * 512 : (nh_ + 1) * 512],
                            wt_sb[k][:, m * P : (m + 1) * P],
                            xs_sb[k][:, nh_ * 512 : (nh_ + 1) * 512],
                            start=(k == 0),
                            stop=(k == 1),
                        )
                if cast_eng == "act":
                    nc.scalar.copy(out=kt[m][:], in_=ps[:])
                else:
                    nc.vector.tensor_copy(out=kt[m][:], in_=ps[:])

            def emit_v2_pair(sp):
                ps = pav.tile([P, S1], F32, tag="pav", name=f"pv2_{sp}")
                for half in range(2):
                    s = 2 * sp + half
                    for k in range(2):
                        nc.tensor.matmul(
                            ps[:, half * 512 : (half + 1) * 512],
                            x2_sb[k][:, s * P : (s + 1) * P],
                            wv2t_sb[k][:],
                            start=(k == 0),
                            stop=(k == 1),
                        )
                for half in range(2):
                    s = 2 * sp + half
                    nc.gpsimd.memset(v2a_sb[s][:, :, VD : VD + 1], 1.0)
                    dst = v2a_sb[s][:, :, 0:VD]
                    src = ps[:, half * 512 : (half + 1) * 512].rearrange(
                        "p (h c) -> p h c", c=VD
                    )
                    if half == 0:
                        nc.scalar.copy(out=dst, in_=src)
                    else:
                        nc.vector.tensor_copy(out=dst, in_=src)

            # ---- prologue: K-chunk 0 projections ----
            emit_proj_chunk(wk1t_sb, x1_sb, k1t_sb, 0, "vec")
            emit_proj_chunk(wk2t_sb, x2_sb, k2t_sb, 0, "act")

            av_tiles = {}
            et_tiles = {}
            pending = []

            def emit_av(c, s2):
                a, b = 2 * c, 2 * c + 1
                if s2 == 0:
                    for h in (a, b):
                        av_tiles[h] = pav.tile(
                            [VD + 1, S1], F32, tag="pav", name=f"av_{h}"
                        )
                for idx, h in enumerate((a, b)):
                    et = et_tiles.pop((c, s2, idx))
                    for nh_ in range(2):
                        nc.tensor.matmul(
                            av_tiles[h][:, nh_ * 512 : (nh_ + 1) * 512],
                            v2a_sb[s2][:, h, :],
                            et[:, nh_ * 512 : (nh_ + 1) * 512],
                            start=(s2 == 0),
                            stop=(s2 == 7),
                            skip_group_check=True,
                        )

            def emit_normalize(cpair):
                a, b = 2 * cpair, 2 * cpair + 1
                avs = {}
                rcps = {}
                for i, h in enumerate((a, b)):
                    # bf16 copy of AV out of PSUM (for the normalize multiply)
                    avs[h] = npool.tile([VD + 1, S1], BF16, tag="avs", name=f"avs_{h}")
                    nc.scalar.copy(out=avs[h][:], in_=av_tiles[h][:])
                    # fp32 reciprocal straight from PSUM (custom DVE needs
                    # partition base 0; av tiles start at 0)
                    rcps[h] = rpool.tile([VD + 1, S1], F32, tag="rcp", name=f"rcp_{h}")
                    nc.vector.reciprocal_approx_fast(rcps[h][:], av_tiles[h][:])
                # partition-broadcast the reciprocal rows via a DRAM bounce
                reps = {}
                for i, h in enumerate((a, b)):
                    reps[h] = npool.tile([64, S1], F32, tag="rep", name=f"rep_{h}")
                    rdram = dpool.tile([S1], F32, tag="rd", name=f"rd_{h}")
                    q = nc.sync if i == 0 else nc.gpsimd
                    q.dma_start(rdram[:], rcps[h][VD : VD + 1, :])
                    q.dma_start(reps[h][:], rdram[None, :].to_broadcast((64, S1)))
                # oall_h = avs_h[0:64] * rep_h
                for i, h in enumerate((a, b)):
                    if h < 6:
                        nc.gpsimd.tensor_mul(
                            out=oall_sb[h][:], in0=avs[h][0:VD, :], in1=reps[h][:]
                        )
                    else:
                        nc.vector.tensor_mul(
                            out=oall_sb[h][:], in0=avs[h][0:VD, :], in1=reps[h][:]
                        )

            def flush_av(upto):
                while len(pending) > upto:
                    cc, ss = pending.pop(0)
                    emit_av(cc, ss)
                    if ss == 7:
                        emit_normalize(cc)
                        if cc + 2 <= 3:
                            emit_proj_chunk(
                                wk1t_sb, x1_sb, k1t_sb, cc + 2,
                                "act" if cc == 0 else "vec",
                            )
                            emit_proj_chunk(
                                wk2t_sb, x2_sb, k2t_sb, cc + 2,
                                "vec" if cc == 0 else "act",
                            )

            def emit_exp(c, s2, idx, qk, eng):
                et = epool.tile([P, S1], BF16, tag="expt", name=f"et_{c}_{s2}_{idx}")
                if eng == "A":
                    nc.scalar.activation(
                        et[:],
                        qk[:],
                        mybir.ActivationFunctionType.Exp,
                        scale=0.125,
                    )
                else:
                    nc.vector._custom_dve(
                        EXP_OP, out=et[:], in0=qk[:], s0=PC1, s1=PC2, imm2=PC3
                    )
                et_tiles[(c, s2, idx)] = et

            for c in range(4):
                for s2 in range(8):
                    # AV for older steps first: keeps the PE busy while the
                    # exp pipeline catches up, so QK never stalls the array.
                    flush_av(2 if c == 0 else 1)
                    qks = {}
                    for idx in range(2):  # head idx within the pair
                        ro = idx * 64
                        qk = pmm.tile(
                            [P, S1], F32, tag="qk", name=f"qk_{c}_{s2}_{idx}"
                        )
                        for nh_ in range(2):
                            nc.tensor.matmul(
                                qk[:, nh_ * 512 : (nh_ + 1) * 512],
                                k2t_sb[c][ro : ro + 64, s2 * P : (s2 + 1) * P],
                                k1t_sb[c][ro : ro + 64, nh_ * 512 : (nh_ + 1) * 512],
                                start=True,
                                stop=True,
                            )
                        qks[idx] = qk
                    if c == 0:
                        if s2 == 0:
                            emit_v2_pair(0)
                            emit_v2_pair(1)
                        elif s2 == 1:
                            emit_v2_pair(2)
                            emit_v2_pair(3)
                        elif s2 == 2:
                            emit_proj_chunk(wk1t_sb, x1_sb, k1t_sb, 1, "vec")
                            emit_proj_chunk(wk2t_sb, x2_sb, k2t_sb, 1, "act")
                    # engine split: alternate per step; one double-ACT step per
                    # pair to balance ACT/DVE per-tile costs
                    if s2 == 3:
                        engs = ("A", "A")
                    elif (s2 + c) % 2 == 0:
                        engs = ("A", "D")
                    else:
                        engs = ("D", "A")
                    for idx in range(2):
                        emit_exp(c, s2, idx, qks[idx], engs[idx])
                    pending.append((c, s2))
            flush_av(0)

            # ---- final projection: y[mt] = sum_h WoT_h.T @ oall_h ----
            fins = {
                mt: pmm.tile([P, S1], F32, tag="qk", name=f"fin_{mt}")
                for mt in range(2)
            }

            def fin_mms(mt, hs):
                for h in hs:
                    for nh_ in range(2):
                        nc.tensor.matmul(
                            fins[mt][:, nh_ * 512 : (nh_ + 1) * 512],
                            wot_sb[h][:, mt * P : (mt + 1) * P],
                            oall_sb[h][:, nh_ * 512 : (nh_ + 1) * 512],
                            start=(h == 0),
                            stop=(h == NH - 1),
                            skip_group_check=True,
                        )

            def ship_y(mt, eng):
                ysb = ypool.tile([P, S1], F32, tag="y", name=f"y_{mt}")
                if eng == "act":
                    nc.scalar.copy(out=ysb[:], in_=fins[mt][:])
                else:
                    nc.vector.tensor_copy(out=ysb[:], in_=fins[mt][:])
                nc.sync.dma_start(y[mt * P : (mt + 1) * P, :], ysb[:])

            fin_mms(0, range(6))
            fin_mms(1, range(6))
            fin_mms(0, (6, 7))
            ship_y(0, "act")
            fin_mms(1, (6, 7))
            ship_y(1, "vec")

    nc.compile()
    return nc


_nc_cache = None


def _get_nc():
    global _nc_cache
    if _nc_cache is None:
        _nc_cache = build_nc()
    return _nc_cache


def _make_in_maps(input1, input2, Wk1, Wk2, Wv2, Wo):
    import ml_dtypes

    bf16 = ml_dtypes.bfloat16
    input1 = np.asarray(input1, dtype=np.float32).astype(bf16)
    input2 = np.asarray(input2, dtype=np.float32).astype(bf16)
    wkv = np.ascontiguousarray(
        np.stack(
            [np.asarray(W, dtype=np.float32).T.astype(bf16) for W in (Wk1, Wk2, Wv2)]
        )
    )
    wot = np.ascontiguousarray(np.asarray(Wo, dtype=np.float32).T.astype(bf16))
    return [
        {
            "x1": np.ascontiguousarray(input1[b].reshape(C1, S1)),
            "x2": np.ascontiguousarray(input2[b].reshape(C2V, S2)),
            "wkv": wkv,
            "wot": wot,
        }
        for b in range(B)
    ]


def _assemble(results):
    out = np.stack([results[b]["y"] for b in range(B)], axis=0)
    return np.ascontiguousarray(out.reshape(B, C1, 32, 32).astype(np.float32))


def kernel(input1, input2, Wk1, Wk2, Wv2, Wo):
    nc = _get_nc()
    in_maps = _make_in_maps(input1, input2, Wk1, Wk2, Wv2, Wo)
    res = bass_utils.run_bass_kernel_spmd(nc, in_maps, core_ids=list(range(B)))
    return _assemble(res.results)


def kernel_traced(input1, input2, Wk1, Wk2, Wv2, Wo):
    """Like kernel() but with NTFF profiling; returns (out, BassKernelResults)."""
    nc = _get_nc()
    in_maps = _make_in_maps(input1, input2, Wk1, Wk2, Wv2, Wo)
    res = bass_utils.run_bass_kernel_spmd(
        nc, in_maps, core_ids=list(range(B)), trace=True
    )
    return _assemble(res.results), res


# revision 8
# speedup vs baseline: 1.1855x; 1.0143x over previous
"""Cross-attention kernel for TRN2, batch-parallel over 8 NeuronCores. v3.

Problem shapes (hardcoded): B=8, C1=C2=256, H=W=32 (S=1024), NH=8, KD=VD=64.

Per-core program (core b computes batch element b, no collectives):
  K1T = Wk1 @ X1  [512, S1] bf16, K2T likewise; V2 per-head with ones column.
  Attention pair-packed over head pairs (2c, 2c+1): QK matmuls bf16 with the
  head pair on disjoint 64-partition ranges (enables PE row-tile co-issue),
  scores psum held in [128, 1024] tiles so each exp instruction covers a full
  head step (amortizes ACT/DVE init overhead).  exp split between ACT (LUT)
  and DVE (custom cubic EXP_CUBIC_ANT).  AV bf16 [65, S1] with ones-column
  denominator.  Normalize: bf16 avs copy (ACT) + fp32 reciprocal from PSUM
  (DVE) + DRAM-bounce broadcast DMA + GPSIMD/DVE multiplies -> bf16 oall.
  Final projection bf16; AV for step s-1 is emitted before QK for step s so
  the PE never stalls on the exp pipeline.
"""

import sys

for _p in ("/opt/trn_rl_repo", "/root/.axon_site/_ro/trn_rl_repo"):
    if _p not in sys.path:
        sys.path.append(_p)

import numpy as np

import concourse.bass as bass
import concourse.mybir as mybir
import concourse.tile as tile
from concourse import bacc, bass_utils
from concourse import dve_ops
from concourse.dve_spec import AluOp, Bin, One, Spec, Src0
from concourse.dve_spec import C0 as SC0, C1 as SC1, C2 as SC2

F32 = mybir.dt.float32
BF16 = mybir.dt.bfloat16

B = 8
C1 = 256
S1 = 1024
C2V = 256
S2 = 1024
NH = 8
KD = 64
VD = 64
P = 128

# ---- custom DVE op: one-pass cubic exp(s/8) ---------------------------------
_EXP_BODY = Bin(
    AluOp.ADD,
    Bin(
        AluOp.MULTIPLY,
        Bin(
            AluOp.ADD,
            Bin(
                AluOp.MULTIPLY,
                Bin(AluOp.ADD, Bin(AluOp.MULTIPLY, Src0, SC2), SC1),
                Src0,
            ),
            SC0,
        ),
        Src0,
    ),
    One,
)
PC1 = 1.00620561 / 8.0
PC2 = 0.51886828 / 64.0
PC3 = 0.15441254 / 512.0


def _exp_ref(in0, in1, s0, s1, imm2):
    return ((in0 * imm2 + s1) * in0 + s0) * in0 + 1.0


EXP_OP = dve_ops.DveOp(
    "EXP_CUBIC_ANT",
    Spec(body=_EXP_BODY, reference=_exp_ref),
    subdim=False,
    uops_sha={"v3": "2b376e79438849ef", "v4": "452976e68db449e6"},
)

if EXP_OP.name not in dve_ops._SUB_OPCODE_FOR_NAME:
    dve_ops.OPS.append(EXP_OP)
    dve_ops.CUSTOM_DVE_SPECS[EXP_OP.name] = EXP_OP.spec
    dve_ops._SUB_OPCODE_FOR_NAME[EXP_OP.name] = (
        dve_ops._CUSTOM_DVE_ROW_BASE + len(dve_ops.OPS) - 1
    )

N_WARM_MM = 8


def build_nc(dump=False):
    nc = bacc.Bacc(
        "TRN2",
        target_bir_lowering=False,
        debug=False,
        enable_asserts=False,
        num_devices=B,
    )

    x1 = nc.dram_tensor("x1", [C1, S1], BF16, kind="ExternalInput").ap()
    x2 = nc.dram_tensor("x2", [C2V, S2], BF16, kind="ExternalInput").ap()
    wkv = nc.dram_tensor("wkv", [3, C1, NH * KD], BF16, kind="ExternalInput").ap()
    wot = nc.dram_tensor("wot", [NH * VD, C1], BF16, kind="ExternalInput").ap()
    y = nc.dram_tensor("y", [C1, S1], F32, kind="ExternalOutput").ap()

    with tile.TileContext(nc) as tc:
        with (
            tc.tile_pool(name="const", bufs=1) as cpool,
            tc.tile_pool(name="expt", bufs=8) as epool,
            tc.tile_pool(name="norm", bufs=4) as npool,
            tc.tile_pool(name="rcpp", bufs=4) as rpool,
            tc.tile_pool(name="yout", bufs=2) as ypool,
            tc.tile_pool(name="pmm", bufs=2, space="PSUM") as pmm,
            tc.tile_pool(name="pav", bufs=2, space="PSUM") as pav,
            tc.tile_pool(name="dscr", bufs=4, space="DRAM") as dpool,
        ):
            # ---- load inputs (parallel queues; wk1 lands first) ----
            x1_big = cpool.tile([P, 2, S1], BF16, name="x1_big")
            x2_big = cpool.tile([P, 2, S2], BF16, name="x2_big")
            wkv_sb = cpool.tile([P, 3, 2, 512], BF16, name="wkv_sb")
            wot_big = cpool.tile([64, NH, C1], BF16, name="wot_big")
            wkv_r = wkv.rearrange("t (c p) f -> p t c f", p=P)
            nc.sync.dma_start(wkv_sb[:, 0], wkv_r[:, 0])
            nc.sync.dma_start(x1_big[:], x1.rearrange("(c p) s -> p c s", p=P))
            nc.gpsimd.dma_start(x2_big[:], x2.rearrange("(c p) s -> p c s", p=P))
            nc.scalar.dma_start(wkv_sb[:, 1], wkv_r[:, 1])
            nc.gpsimd.dma_start(wkv_sb[:, 2], wkv_r[:, 2])
            nc.sync.dma_start(wot_big[:], wot.rearrange("(h r) c -> r h c", r=64))

            x1_sb = [x1_big[:, c, :] for c in range(2)]
            x2_sb = [x2_big[:, c, :] for c in range(2)]
            wk1t_sb = [wkv_sb[:, 0, c, :] for c in range(2)]
            wk2t_sb = [wkv_sb[:, 1, c, :] for c in range(2)]
            wv2t_sb = [wkv_sb[:, 2, c, :] for c in range(2)]
            wot_sb = [wot_big[:, h, :] for h in range(NH)]

            k1t_sb = [cpool.tile([P, S1], BF16, name=f"k1t_{m}") for m in range(4)]
            k2t_sb = [cpool.tile([P, S2], BF16, name=f"k2t_{m}") for m in range(4)]
            v2a_sb = [
                cpool.tile([P, NH, VD + 1], BF16, name=f"v2a_{s}") for s in range(8)
            ]
            oall_sb = [cpool.tile([64, S1], BF16, name=f"oall_{h}") for h in range(NH)]

            # ---- warmups: ACT exp table + PE HAM release ----
            warm_sb = cpool.tile([P, 512], BF16, name="warm_sb")
            warm_out = cpool.tile([1, 64], BF16, name="warm_out")
            nc.gpsimd.memset(warm_sb[:], 0.0)
            nc.scalar.activation(
                warm_out[:], warm_sb[0:1, 0:64], mybir.ActivationFunctionType.Exp
            )
            if N_WARM_MM:
                wps = pmm.tile([P, S1], F32, tag="qk", name="warm_ps")
                for i in range(N_WARM_MM):
                    nc.tensor.matmul(
                        wps[:, (i % 2) * 512 : (i % 2) * 512 + 512],
                        warm_sb[:, 0:128],
                        warm_sb[:],
                        start=True,
                        stop=True,
                        skip_group_check=True,
                    )

            def emit_proj_chunk(wt_sb, xs_sb, kt, m, cast_eng):
                """kt[m] (bf16 [128, S]) = (wt chunk).T @ xs.  k-outer order so
                the stationary (wt chunk) is loaded once per k."""
                ps = pav.tile([P, S1], F32, tag="pav", name=f"pj_{kt[m].name}")
                for k in range(2):
                    for nh_ in range(2):
                        nc.tensor.matmul(
                            ps[:, nh_ * 512 : (nh_ + 1) * 512],
                            wt_sb[k][:, m * P : (m + 1) * P],
                            xs_sb[k][:, nh_ * 512 : (nh_ + 1) * 512],
                            start=(k == 0),
                            stop=(k == 1),
                        )
                if cast_eng == "act":
                    nc.scalar.copy(out=kt[m][:], in_=ps[:])
                else:
                    nc.vector.tensor_copy(out=kt[m][:], in_=ps[:])

            def emit_v2_pair(sp):
                ps = pav.tile([P, S1], F32, tag="pav", name=f"pv2_{sp}")
                for half in range(2):
                    s = 2 * sp + half
                    for k in range(2):
                        nc.tensor.matmul(
                            ps[:, half * 512 : (half + 1) * 512],
                            x2_sb[k][:, s * P : (s + 1) * P],
                            wv2t_sb[k][:],
                            start=(k == 0),
                            stop=(k == 1),
                        )
                for half in range(2):
                    s = 2 * sp + half
                    nc.gpsimd.memset(v2a_sb[s][:, :, VD : VD + 1], 1.0)
                    dst = v2a_sb[s][:, :, 0:VD]
                    src = ps[:, half * 512 : (half + 1) * 512].rearrange(
                        "p (h c) -> p h c", c=VD
                    )
                    if half == 0:
                        nc.scalar.copy(out=dst, in_=src)
                    else:
                        nc.vector.tensor_copy(out=dst, in_=src)

            # ---- prologue: K-chunk 0 projections ----
            emit_proj_chunk(wk1t_sb, x1_sb, k1t_sb, 0, "vec")
            emit_proj_chunk(wk2t_sb, x2_sb, k2t_sb, 0, "act")

            av_tiles = {}
            et_tiles = {}
            pending = []

            def emit_av(c, s2):
                a, b = 2 * c, 2 * c + 1
                if s2 == 0:
                    for h in (a, b):
                        av_tiles[h] = pav.tile(
                            [VD + 1, S1], F32, tag="pav", name=f"av_{h}"
                        )
                for idx, h in enumerate((a, b)):
                    et = et_tiles.pop((c, s2, idx))
                    for nh_ in range(2):
                        nc.tensor.matmul(
                            av_tiles[h][:, nh_ * 512 : (nh_ + 1) * 512],
                            v2a_sb[s2][:, h, :],
                            et[:, nh_ * 512 : (nh_ + 1) * 512],
                            start=(s2 == 0),
                            stop=(s2 == 7),
                            skip_group_check=True,
                        )

            def emit_normalize(cpair):
                a, b = 2 * cpair, 2 * cpair + 1
                avs = {}
                rcps = {}
                for i, h in enumerate((a, b)):
                    # fp32 copy of AV out of PSUM; the reciprocal reads this
                    # copy so the PSUM slot frees as soon as the copy lands
                    avs[h] = npool.tile([VD + 1, S1], F32, tag="avs", name=f"avs_{h}")
                    nc.scalar.copy(out=avs[h][:], in_=av_tiles[h][:])
                    rcps[h] = rpool.tile([VD + 1, S1], F32, tag="rcp", name=f"rcp_{h}")
                    nc.vector.reciprocal_approx_fast(rcps[h][:], avs[h][:])
                # partition-broadcast the reciprocal rows via a DRAM bounce
                reps = {}
                for i, h in enumerate((a, b)):
                    reps[h] = npool.tile([64, S1], F32, tag="rep", name=f"rep_{h}")
                    rdram = dpool.tile([S1], F32, tag="rd", name=f"rd_{h}")
                    q = nc.sync if i == 0 else nc.gpsimd
                    q.dma_start(rdram[:], rcps[h][VD : VD + 1, :])
                    q.dma_start(reps[h][:], rdram[None, :].to_broadcast((64, S1)))
                # oall_h = avs_h[0:64] * rep_h
                for i, h in enumerate((a, b)):
                    if h < 6:
                        nc.gpsimd.tensor_mul(
                            out=oall_sb[h][:], in0=avs[h][0:VD, :], in1=reps[h][:]
                        )
                    else:
                        nc.vector.tensor_mul(
                            out=oall_sb[h][:], in0=avs[h][0:VD, :], in1=reps[h][:]
                        )

            def flush_av(upto):
                while len(pending) > upto:
                    cc, ss = pending.pop(0)
                    emit_av(cc, ss)
                    if ss == 7:
                        emit_normalize(cc)
                        if cc + 2 <= 3:
                            emit_proj_chunk(
                                wk1t_sb, x1_sb, k1t_sb, cc + 2,
                                "act" if cc == 0 else "vec",
                            )
                            emit_proj_chunk(
                                wk2t_sb, x2_sb, k2t_sb, cc + 2,
                                "vec" if cc == 0 else "act",
                            )

            def emit_exp(c, s2, idx, qk, eng):
                et = epool.tile([P, S1], BF16, tag="expt", name=f"et_{c}_{s2}_{idx}")
                if eng == "S":
                    # tail latency: split across both engines
                    nc.scalar.activation(
                        et[:, 0:512],
                        qk[:, 0:512],
                        mybir.ActivationFunctionType.Exp,
                        scale=0.125,
                    )
                    nc.vector._custom_dve(
                        EXP_OP, out=et[:, 512:1024], in0=qk[:, 512:1024],
                        s0=PC1, s1=PC2, imm2=PC3,
                    )
                elif eng == "A":
                    nc.scalar.activation(
                        et[:],
                        qk[:],
                        mybir.ActivationFunctionType.Exp,
                        scale=0.125,
                    )
                else:
                    nc.vector._custom_dve(
                        EXP_OP, out=et[:], in0=qk[:], s0=PC1, s1=PC2, imm2=PC3
                    )
                et_tiles[(c, s2, idx)] = et

            for c in range(4):
                for s2 in range(8):
                    # AV for older steps first: keeps the PE busy while the
                    # exp pipeline catches up, so QK never stalls the array.
                    flush_av(2 if c == 0 else 1)
                    # interleave the head pair: adjacent matmuls sit on
                    # disjoint 64-partition row ranges so the PE can co-issue
                    qks = {
                        idx: pmm.tile([P, S1], F32, tag="qk", name=f"qk_{c}_{s2}_{idx}")
                        for idx in range(2)
                    }
                    for nh_ in range(2):
                        for idx in range(2):
                            ro = idx * 64
                            nc.tensor.matmul(
                                qks[idx][:, nh_ * 512 : (nh_ + 1) * 512],
                                k2t_sb[c][ro : ro + 64, s2 * P : (s2 + 1) * P],
                                k1t_sb[c][ro : ro + 64, nh_ * 512 : (nh_ + 1) * 512],
                                start=True,
                                stop=True,
                            )
                    if c == 0:
                        if s2 == 0:
                            emit_v2_pair(0)
                            emit_v2_pair(1)
                        elif s2 == 1:
                            emit_v2_pair(2)
                            emit_v2_pair(3)
                        elif s2 == 2:
                            emit_proj_chunk(wk1t_sb, x1_sb, k1t_sb, 1, "vec")
                            emit_proj_chunk(wk2t_sb, x2_sb, k2t_sb, 1, "act")
                    # engine split: alternate per step; one double-ACT step per
                    # pair to balance ACT/DVE per-tile costs; last step of the
                    # kernel splits each tile across engines for tail latency
                    if c == 3 and s2 == 7:
                        engs = ("S", "S")
                    elif s2 == 3:
                        engs = ("A", "A")
                    elif (s2 + c) % 2 == 0:
                        engs = ("A", "D")
                    else:
                        engs = ("D", "A")
                    for idx in range(2):
                        emit_exp(c, s2, idx, qks[idx], engs[idx])
                    pending.append((c, s2))
            flush_av(0)

            # ---- final projection: y[mt] = sum_h WoT_h.T @ oall_h ----
            fins = {
                mt: pmm.tile([P, S1], F32, tag="qk", name=f"fin_{mt}")
                for mt in range(2)
            }

            def fin_mms(mt, hs):
                for h in hs:
                    for nh_ in range(2):
                        nc.tensor.matmul(
                            fins[mt][:, nh_ * 512 : (nh_ + 1) * 512],
                            wot_sb[h][:, mt * P : (mt + 1) * P],
                            oall_sb[h][:, nh_ * 512 : (nh_ + 1) * 512],
                            start=(h == 0),
                            stop=(h == NH - 1),
                            skip_group_check=True,
                        )

            def ship_y(mt, eng):
                ysb = ypool.tile([P, S1], F32, tag="y", name=f"y_{mt}")
                if eng == "act":
                    nc.scalar.copy(out=ysb[:], in_=fins[mt][:])
                else:
                    nc.vector.tensor_copy(out=ysb[:], in_=fins[mt][:])
                nc.sync.dma_start(y[mt * P : (mt + 1) * P, :], ysb[:])

            fin_mms(0, range(6))
            fin_mms(1, range(6))
            fin_mms(0, (6, 7))
            ship_y(0, "act")
            fin_mms(1, (6, 7))
            ship_y(1, "vec")

    nc.compile()
    return nc


_nc_cache = None


def _get_nc():
    global _nc_cache
    if _nc_cache is None:
        _nc_cache = build_nc()
    return _nc_cache


def _make_in_maps(input1, input2, Wk1, Wk2, Wv2, Wo):
    import ml_dtypes

    bf16 = ml_dtypes.bfloat16
    input1 = np.asarray(input1, dtype=np.float32).astype(bf16)
    input2 = np.asarray(input2, dtype=np.float32).astype(bf16)
    wkv = np.ascontiguousarray(
        np.stack(
            [np.asarray(W, dtype=np.float32).T.astype(bf16) for W in (Wk1, Wk2, Wv2)]
        )
    )
    wot = np.ascontiguousarray(np.asarray(Wo, dtype=np.float32).T.astype(bf16))
    return [
        {
            "x1": np.ascontiguousarray(input1[b].reshape(C1, S1)),
            "x2": np.ascontiguousarray(input2[b].reshape(C2V, S2)),
            "wkv": wkv,
            "wot": wot,
        }
        for b in range(B)
    ]


def _assemble(results):
    out = np.stack([results[b]["y"] for b in range(B)], axis=0)
    return np.ascontiguousarray(out.reshape(B, C1, 32, 32).astype(np.float32))


def kernel(input1, input2, Wk1, Wk2, Wv2, Wo):
    nc = _get_nc()
    in_maps = _make_in_maps(input1, input2, Wk1, Wk2, Wv2, Wo)
    res = bass_utils.run_bass_kernel_spmd(nc, in_maps, core_ids=list(range(B)))
    return _assemble(res.results)


def kernel_traced(input1, input2, Wk1, Wk2, Wv2, Wo):
    """Like kernel() but with NTFF profiling; returns (out, BassKernelResults)."""
    nc = _get_nc()
    in_maps = _make_in_maps(input1, input2, Wk1, Wk2, Wv2, Wo)
    res = bass_utils.run_bass_kernel_spmd(
        nc, in_maps, core_ids=list(range(B)), trace=True
    )
    return _assemble(res.results), res
